# revision 5
# baseline (speedup 1.0000x reference)
"""Trainium2 Bass kernel for nn_ContrastiveCRFLoss (self-contained).

Math: for each batch b and sample pairs (n, m) over 2048 gathered pixels:
    out[b,n,m] = -(C[b,n,m] * (W1*exp(-cd - gd[b]/(2*BETA)) + W2*exp(-cd/(2*GAMMA))))
where C = cluster Gram, cd = squared coord distance, gd = squared guidance
distance.

Device strategy (8 cores, grid-parallel over the n-rows):
  - Each core owns a 256-row block of the 2048x2048 pair grid, all 8 batches.
  - Three small-K fp16 matmuls per output tile, packed into PE row groups at
    partitions 0 / 32 / 64:
      group0 K=27: pC  = (-clusters)^T clusters            (negated Gram)
      group1 K=9 : p1  = full argument of the first exp    (augmented Gram)
      group2 K=12: p2  = full argument of the second exp   (coord-only, shared
                                                            across batches)
  - ACT: e1 = exp(p1) -> fp16 SBUF at N=2048 (PSUM p1 spans 4 banks).
    pC lands in 2 rotating [128,1024] PSUM tiles (the other 4 banks).
  - DVE: s = e1 + e2 at N=2048 fp16 (2x mode) for a subset of tiles; GpSimd
    handles the rest of the adds.  DVE does all multiplies o = pC * s from
    PSUM (1x mode, N=1024 per pC half), writing fp16.
  - Output fp16 [MT, 128, B*NS]: batches adjacent in the free dim so a
    two-batch [128, 2*NS] DMA writes 8KB contiguous per partition row.
"""

import numpy as np

import concourse.bass as bass
import concourse.mybir as mybir
import concourse.bass_utils as bass_utils
from concourse.tile import TileContext
from concourse.vector_clock import ScopedClock

F16 = mybir.dt.float16
F32 = mybir.dt.float32

# problem constants (hardcoded per the task contract)
ALPHA, BETA, GAMMA = 0.5, 0.15, 25.0
W1, W2, SHIFT = 10.0, 3.0, 0.0
B, CG, CC, H = 8, 3, 27, 224
NS = 2048  # samples
NCORES = 8
MT = 2  # 128-row M-tiles per core
KC, K1, K2 = 27, 9, 12
HN = NS // 2  # 1024

# half-tiles (h) whose e1+e2 add runs on GpSimd instead of DVE: GpSimd owns
# h=0 of every tile, DVE owns h=1 plus all multiplies
def _add_on_gps(t, h):
    return h == 0

# ---------------------------------------------------------------------------
# Walrus in this image rejects >1 sync wait per instruction. Split the Tile
# tail-drain's waits and any multi-wait instruction into single-wait NOPs.
# ---------------------------------------------------------------------------
_MAXW = 1


def _split_drain_and_barrier(self, tick_clock, wait_clock):
    probe = self.nc.sync.nop(nofuse=True)
    wait_clock.add_sem_waits(probe.ins, ScopedClock({None: tick_clock.global_clock}))
    si = probe.ins.sync_info
    waits = list(si.on_wait)
    probe.ins.sync_info = mybir.SyncInfo(
        on_wait=waits[:_MAXW], on_update=list(si.on_update)
    )
    for i in range(_MAXW, len(waits), _MAXW):
        n2 = self.nc.sync.nop(nofuse=True)
        n2.ins.sync_info = mybir.SyncInfo(on_wait=waits[i : i + _MAXW], on_update=[])
    self.nc.sync.drain()
    self.nc.all_engine_barrier()
    popped = self.nc._tile_sem_poison_stack.pop()
    assert popped is self._sem_poison
    self.nc.clear_and_free_semaphores(list(self.sems.allocated().values()))
    self.nc.all_engine_barrier()


def _split_multiwait_insts(nc):
    n_split = 0
    for fn in nc.m.functions:
        for bb in fn.blocks:
            insts = list(bb.instructions)
            new_insts = []
            changed = False
            for inst in insts:
                si = inst.sync_info
                waits = list(si.on_wait) if si is not None else []
                if len(waits) > _MAXW:
                    n_split += 1
                    changed = True
                    n_extra = len(waits) - _MAXW
                    for i in range(0, n_extra, _MAXW):
                        nop = mybir.InstNoOp(
                            name=nc.get_next_instruction_name(),
                            engine=inst.engine,
                            bass_nofuse=True,
                            sync_info=mybir.SyncInfo(
                                on_wait=waits[i : i + _MAXW], on_update=[]
                            ),
                        )
                        new_insts.append(nop)
                    inst.sync_info = mybir.SyncInfo(
                        on_wait=waits[n_extra:], on_update=list(si.on_update)
                    )
                new_insts.append(inst)
            if changed:
                bb.instructions = new_insts
    return n_split


def _install_tile_patch():
    TileContext._drain_and_barrier = _split_drain_and_barrier


# ---------------------------------------------------------------------------
# Device program (identical on all cores; data differs per core)
# ---------------------------------------------------------------------------

def build_nc():
    _install_tile_patch()
    nc = bass.Bass()
    wc = nc.declare_dram_parameter("wc", [KC, MT * B * 128], F16, isOutput=False)
    a1 = nc.declare_dram_parameter("a1", [K1, MT * B * 128], F16, isOutput=False)
    a2 = nc.declare_dram_parameter("a2", [K2, MT * 128], F16, isOutput=False)
    rc = nc.declare_dram_parameter("rc", [KC, B * NS], F16, isOutput=False)
    r1 = nc.declare_dram_parameter("r1", [K1, B * NS], F16, isOutput=False)
    r2 = nc.declare_dram_parameter("r2", [K2, NS], F16, isOutput=False)
    out = nc.declare_dram_parameter("out", [MT, 128, B * NS], F16, isOutput=True)

    with TileContext(nc) as tc:
        with (
            tc.tile_pool(name="w", bufs=1) as wpool,
            tc.tile_pool(name="r", bufs=1) as rpool,
            tc.tile_pool(name="e2p", bufs=2) as e2pool,
            tc.tile_pool(name="e1p", bufs=2) as e1pool,
            tc.tile_pool(name="sp", bufs=2) as spool,
            tc.tile_pool(name="ob", bufs=3) as opool,
            tc.tile_pool(name="pc", bufs=2, space="PSUM") as pcpool,
            tc.tile_pool(name="p1", bufs=2, space="PSUM") as p1pool,
        ):
            W = wpool.tile([128, MT * B * 128], F16)
            R = rpool.tile([128, B * NS], F16)
            nc.sync.dma_start(W[0:KC, :], wc[:])
            nc.sync.dma_start(W[32 : 32 + K1, :], a1[:])
            nc.sync.dma_start(W[64 : 64 + K2, 0 : MT * 128], a2[:])
            nc.sync.dma_start(R[0:KC, :], rc[:])
            nc.sync.dma_start(R[32 : 32 + K1, :], r1[:])
            nc.sync.dma_start(R[64 : 64 + K2, 0:NS], r2[:])

            for m in range(MT):
                # batch-independent second-exp term for this row block
                e2 = e2pool.tile([128, NS], F16, tag="e2")
                for h in range(2):
                    p2 = p1pool.tile([128, HN], F32, tag="p1")
                    for j in range(2):
                        jj = h * 2 + j
                        nc.tensor.matmul(
                            p2[:, j * 512 : (j + 1) * 512],
                            W[64 : 64 + K2, m * 128 : (m + 1) * 128],
                            R[64 : 64 + K2, jj * 512 : (jj + 1) * 512],
                            start=True,
                            stop=True,
                            tile_position=(64, 0),
                        )
                    nc.scalar.activation(
                        e2[:, h * HN : (h + 1) * HN],
                        p2[:],
                        mybir.ActivationFunctionType.Exp,
                    )

                o = None
                for b in range(B):
                    col = (b * MT + m) * 128
                    t = m * B + b
                    e1 = e1pool.tile([128, NS], F16, tag="e1")
                    s = spool.tile([128, NS], F16, tag="s")
                    if b % 2 == 0:
                        o = opool.tile([128, 2 * NS], F16, tag="o")
                    p1h = [
                        p1pool.tile([128, HN], F32, tag="p1", name=f"p1h{h}")
                        for h in range(2)
                    ]
                    pch = [
                        pcpool.tile([128, HN], F32, tag="pc", name=f"pch{h}")
                        for h in range(2)
                    ]
                    # p1 matmuls first (feed ACT early); pC matmuls last so
                    # their PSUM residency before the multiply is short.
                    # Alternate row groups q32/q0 so independent matmuls
                    # overlap in the PE array.
                    for j in range(2):
                        nc.tensor.matmul(
                            p1h[0][:, j * 512 : (j + 1) * 512],
                            W[32 : 32 + K1, col : col + 128],
                            R[32 : 32 + K1, b * NS + j * 512 : b * NS + (j + 1) * 512],
                            start=True,
                            stop=True,
                            tile_position=(32, 0),
                        )
                        nc.tensor.matmul(
                            pch[1][:, j * 512 : (j + 1) * 512],
                            W[0:KC, col : col + 128],
                            R[
                                0:KC,
                                b * NS + (2 + j) * 512 : b * NS + (3 + j) * 512,
                            ],
                            start=True,
                            stop=True,
                            tile_position=(0, 0),
                        )
                    nc.scalar.activation(
                        e1[:, 0:HN], p1h[0][:], mybir.ActivationFunctionType.Exp
                    )
                    for j in range(2):
                        nc.tensor.matmul(
                            p1h[1][:, j * 512 : (j + 1) * 512],
                            W[32 : 32 + K1, col : col + 128],
                            R[
                                32 : 32 + K1,
                                b * NS + (2 + j) * 512 : b * NS + (3 + j) * 512,
                            ],
                            start=True,
                            stop=True,
                            tile_position=(32, 0),
                        )
                        nc.tensor.matmul(
                            pch[0][:, j * 512 : (j + 1) * 512],
                            W[0:KC, col : col + 128],
                            R[0:KC, b * NS + j * 512 : b * NS + (j + 1) * 512],
                            start=True,
                            stop=True,
                            tile_position=(0, 0),
                        )
                    nc.scalar.activation(
                        e1[:, HN:NS], p1h[1][:], mybir.ActivationFunctionType.Exp
                    )
                    oc = (b % 2) * NS
                    for h in range(2):
                        hs = slice(h * HN, (h + 1) * HN)
                        if _add_on_gps(t, h):
                            nc.gpsimd.tensor_add(s[:, hs], e1[:, hs], e2[:, hs])
                        else:
                            nc.vector.tensor_add(s[:, hs], e1[:, hs], e2[:, hs])
                    for h in (1, 0):
                        nc.vector.tensor_tensor(
                            o[:, oc + h * HN : oc + (h + 1) * HN],
                            pch[h][:],
                            s[:, h * HN : (h + 1) * HN],
                            mybir.AluOpType.mult,
                        )
                    if b % 2 == 1:
                        nc.sync.dma_start(
                            out[m, :, (b - 1) * NS : (b + 1) * NS], o[:]
                        )

    _split_multiwait_insts(nc)
    return nc


# ---------------------------------------------------------------------------
# Host-side input prep
# ---------------------------------------------------------------------------

def _f16(x):
    return np.asarray(x, dtype=np.float16)


def _hi_lo(x):
    """Split fp64 vector into two fp16 rows summing to ~x."""
    hi = _f16(x)
    lo = _f16(x - hi.astype(np.float64))
    return hi, lo


def prepare_inputs(guidance, clusters, coords):
    ci = np.asarray(coords[0], dtype=np.int64)
    cj = np.asarray(coords[1], dtype=np.int64)
    # gathers: [B, C, NS]
    sel_g = guidance[:, :, ci, cj].astype(np.float64)
    sel_c = clusters[:, :, ci, cj].astype(np.float32)

    # --- cluster Gram operands (fp16 snap) ---
    c16 = _f16(sel_c)  # [B, 27, NS] rhs
    wc_all = -c16  # lhsT (negated -> folds the leading minus)

    # --- first-exp argument operands ---
    # arg1 = -cd/(2a) - gd/(2beta) + ln(W1) ; 2a = 1
    u16 = _f16(sel_g / np.sqrt(2.0 * BETA))  # [B, 3, NS]
    xc16 = _f16((np.stack([ci, cj]) - 112.0))  # [2, NS] exact
    f1 = (u16.astype(np.float64) ** 2).sum(1) + (
        xc16.astype(np.float64) ** 2
    ).sum(0)  # [B, NS]
    a1_all = np.empty((B, K1, NS), np.float16)
    r1_all = np.empty((B, K1, NS), np.float16)
    ones = np.ones(NS, np.float16)
    for b in range(B):
        b1 = np.log(W1) - f1[b]
        b1h, b1l = _hi_lo(b1)
        f1h, f1l = _hi_lo(f1[b])
        a1_all[b, 0:3] = u16[b]
        a1_all[b, 3:5] = xc16
        a1_all[b, 5] = ones
        a1_all[b, 6] = ones
        a1_all[b, 7] = f1h
        a1_all[b, 8] = f1l
        r1_all[b, 0:3] = _f16(2.0 * u16[b].astype(np.float64))
        r1_all[b, 3:5] = _f16(2.0 * xc16.astype(np.float64))
        r1_all[b, 5] = b1h
        r1_all[b, 6] = b1l
        r1_all[b, 7] = -ones
        r1_all[b, 8] = -ones

    # --- second-exp argument operands (batch independent) ---
    v = (np.stack([ci, cj]) - 112.0) / np.sqrt(2.0 * GAMMA)  # [2, NS] fp64
    vh = _f16(v)
    vl = _f16(v - vh.astype(np.float64))
    vs = vh.astype(np.float64) + vl.astype(np.float64)  # snapped value
    f2 = (vs**2).sum(0)  # [NS]
    b2 = np.log(W2) - f2
    b2h, b2l = _hi_lo(b2)
    f2h, f2l = _hi_lo(f2)
    a2 = np.empty((K2, NS), np.float16)
    r2 = np.empty((K2, NS), np.float16)
    # cross products: (vh+vl)_n * 2*(vh+vl)_m  per dim
    a2[0:2] = vh
    a2[2:4] = vh
    a2[4:6] = vl
    a2[6:8] = vl
    r2[0:2] = _f16(2.0 * vh.astype(np.float64))
    r2[2:4] = _f16(2.0 * vl.astype(np.float64))
    r2[4:6] = _f16(2.0 * vh.astype(np.float64))
    r2[6:8] = _f16(2.0 * vl.astype(np.float64))
    a2[8] = ones
    a2[9] = ones
    a2[10] = f2h
    a2[11] = f2l
    r2[8] = b2h
    r2[9] = b2l
    r2[10] = -ones
    r2[11] = -ones

    # --- per-core input maps (core k owns n-rows [256k, 256k+256)) ---
    in_maps = []
    for k in range(NCORES):
        rows = slice(256 * k, 256 * k + 256)
        # A-side column layouts: [(b * MT + m) * 128] for wc/a1, [m * 128] for a2
        wc_k = wc_all[:, :, rows].transpose(1, 0, 2).reshape(KC, B * 256)
        a1_k = a1_all[:, :, rows].transpose(1, 0, 2).reshape(K1, B * 256)
        a2_k = np.ascontiguousarray(a2[:, rows])
        in_maps.append(
            {
                "wc": np.ascontiguousarray(wc_k),
                "a1": np.ascontiguousarray(a1_k),
                "a2": a2_k,
                "rc": np.ascontiguousarray(c16.transpose(1, 0, 2).reshape(KC, B * NS)),
                "r1": np.ascontiguousarray(
                    r1_all.transpose(1, 0, 2).reshape(K1, B * NS)
                ),
                "r2": r2,
            }
        )
    return in_maps


_NC_CACHE = {}


def _get_nc():
    if "nc" not in _NC_CACHE:
        _NC_CACHE["nc"] = build_nc()
    return _NC_CACHE["nc"]


def kernel(guidance, clusters, coords):
    guidance = np.asarray(guidance)
    clusters = np.asarray(clusters)
    coords = np.asarray(coords)
    in_maps = prepare_inputs(guidance, clusters, coords)
    nc = _get_nc()
    res = bass_utils.run_bass_kernel_spmd(nc, in_maps, list(range(NCORES)))
    # res.results[k]["out"]: [MT, 128, B*NS] fp16 -> rows 256k..256k+256
    full = np.concatenate(
        [
            np.asarray(res.results[k]["out"])
            .reshape(MT, 128, B, NS)
            .transpose(2, 0, 1, 3)
            .reshape(B, MT * 128, NS)
            for k in range(NCORES)
        ],
        axis=1,
    )
    return full.astype(np.float32)


# revision 8
# speedup vs baseline: 1.0636x; 1.0636x over previous
"""Trainium2 Bass kernel for nn_ContrastiveCRFLoss (self-contained).

Math: for each batch b and sample pairs (n, m) over 2048 gathered pixels:
    out[b,n,m] = -(C[b,n,m] * (W1*exp(-cd - gd[b]/(2*BETA)) + W2*exp(-cd/(2*GAMMA))))
where C = cluster Gram, cd = squared coord distance, gd = squared guidance
distance.

Device strategy (8 cores, grid-parallel over the n-rows):
  - Each core owns a 256-row block of the 2048x2048 pair grid, all 8 batches.
  - Three small-K fp16 matmuls per output tile, packed into PE row groups at
    partitions 0 / 32 / 64:
      group0 K=27: pC  = (-clusters)^T clusters            (negated Gram)
      group1 K=9 : p1  = full argument of the first exp    (augmented Gram)
      group2 K=12: p2  = full argument of the second exp   (coord-only, shared
                                                            across batches)
  - ACT: e1 = exp(p1) -> fp16 SBUF at N=2048 (PSUM p1 spans 4 banks).
    pC lands in 2 rotating [128,1024] PSUM tiles (the other 4 banks).
  - DVE: s = e1 + e2 at N=2048 fp16 (2x mode) for a subset of tiles; GpSimd
    handles the rest of the adds.  DVE does all multiplies o = pC * s from
    PSUM (1x mode, N=1024 per pC half), writing fp16.
  - Output fp16 [MT, 128, B*NS]: batches adjacent in the free dim so a
    two-batch [128, 2*NS] DMA writes 8KB contiguous per partition row.
"""

import numpy as np

import concourse.bass as bass
import concourse.mybir as mybir
import concourse.bass_utils as bass_utils
from concourse.tile import TileContext
from concourse.vector_clock import ScopedClock

F16 = mybir.dt.float16
F32 = mybir.dt.float32

# problem constants (hardcoded per the task contract)
ALPHA, BETA, GAMMA = 0.5, 0.15, 25.0
W1, W2, SHIFT = 10.0, 3.0, 0.0
B, CG, CC, H = 8, 3, 27, 224
NS = 2048  # samples
NCORES = 8
MT = 2  # 128-row M-tiles per core
KC, K1, K2 = 27, 9, 12
HN = NS // 2  # 1024

# half-tiles (t, h) whose e1+e2 add runs on GpSimd instead of DVE: GpSimd
# owns h=0 of every tile plus h=1 of four tiles; DVE owns the rest plus all
# multiplies
def _add_on_gps(t, h):
    return h == 0 or t in (2, 6, 10, 14)

# ---------------------------------------------------------------------------
# Walrus in this image rejects >1 sync wait per instruction. Split the Tile
# tail-drain's waits and any multi-wait instruction into single-wait NOPs.
# ---------------------------------------------------------------------------
_MAXW = 1


def _split_drain_and_barrier(self, tick_clock, wait_clock):
    probe = self.nc.sync.nop(nofuse=True)
    wait_clock.add_sem_waits(probe.ins, ScopedClock({None: tick_clock.global_clock}))
    si = probe.ins.sync_info
    waits = list(si.on_wait)
    probe.ins.sync_info = mybir.SyncInfo(
        on_wait=waits[:_MAXW], on_update=list(si.on_update)
    )
    for i in range(_MAXW, len(waits), _MAXW):
        n2 = self.nc.sync.nop(nofuse=True)
        n2.ins.sync_info = mybir.SyncInfo(on_wait=waits[i : i + _MAXW], on_update=[])
    self.nc.sync.drain()
    self.nc.all_engine_barrier()
    popped = self.nc._tile_sem_poison_stack.pop()
    assert popped is self._sem_poison
    self.nc.clear_and_free_semaphores(list(self.sems.allocated().values()))
    self.nc.all_engine_barrier()


def _split_multiwait_insts(nc):
    n_split = 0
    for fn in nc.m.functions:
        for bb in fn.blocks:
            insts = list(bb.instructions)
            new_insts = []
            changed = False
            for inst in insts:
                si = inst.sync_info
                waits = list(si.on_wait) if si is not None else []
                if len(waits) > _MAXW:
                    n_split += 1
                    changed = True
                    n_extra = len(waits) - _MAXW
                    for i in range(0, n_extra, _MAXW):
                        nop = mybir.InstNoOp(
                            name=nc.get_next_instruction_name(),
                            engine=inst.engine,
                            bass_nofuse=True,
                            sync_info=mybir.SyncInfo(
                                on_wait=waits[i : i + _MAXW], on_update=[]
                            ),
                        )
                        new_insts.append(nop)
                    inst.sync_info = mybir.SyncInfo(
                        on_wait=waits[n_extra:], on_update=list(si.on_update)
                    )
                new_insts.append(inst)
            if changed:
                bb.instructions = new_insts
    return n_split


def _install_tile_patch():
    TileContext._drain_and_barrier = _split_drain_and_barrier


# ---------------------------------------------------------------------------
# Device program (identical on all cores; data differs per core)
# ---------------------------------------------------------------------------

def build_nc():
    _install_tile_patch()
    nc = bass.Bass()
    wc = nc.declare_dram_parameter("wc", [KC, MT * B * 128], F16, isOutput=False)
    a1 = nc.declare_dram_parameter("a1", [K1, MT * B * 128], F16, isOutput=False)
    a2 = nc.declare_dram_parameter("a2", [K2, MT * 128], F16, isOutput=False)
    rc = nc.declare_dram_parameter("rc", [KC, B * NS], F16, isOutput=False)
    r1 = nc.declare_dram_parameter("r1", [K1, B * NS], F16, isOutput=False)
    r2 = nc.declare_dram_parameter("r2", [K2, NS], F16, isOutput=False)
    out = nc.declare_dram_parameter("out", [MT, 128, B * NS], F16, isOutput=True)

    with TileContext(nc) as tc:
        with (
            tc.tile_pool(name="w", bufs=1) as wpool,
            tc.tile_pool(name="r", bufs=1) as rpool,
            tc.tile_pool(name="e2p", bufs=2) as e2pool,
            tc.tile_pool(name="e1p", bufs=3) as e1pool,
            tc.tile_pool(name="sp", bufs=3) as spool,
            tc.tile_pool(name="ob", bufs=3) as opool,
            tc.tile_pool(name="pc", bufs=2, space="PSUM") as pcpool,
            tc.tile_pool(name="p1", bufs=2, space="PSUM") as p1pool,
        ):
            W = wpool.tile([128, MT * B * 128], F16)
            R = rpool.tile([128, B * NS], F16)
            nc.sync.dma_start(W[0:KC, :], wc[:])
            nc.sync.dma_start(W[32 : 32 + K1, :], a1[:])
            nc.sync.dma_start(W[64 : 64 + K2, 0 : MT * 128], a2[:])
            nc.sync.dma_start(R[0:KC, :], rc[:])
            nc.sync.dma_start(R[32 : 32 + K1, :], r1[:])
            nc.sync.dma_start(R[64 : 64 + K2, 0:NS], r2[:])

            NT_TILES = MT * B  # 16

            def emit_e2(m):
                """p2 matmuls + exps for the batch-independent term of m."""
                e2 = e2pool.tile([128, NS], F16, tag="e2")
                for h in range(2):
                    p2 = p1pool.tile([128, HN], F32, tag="p1")
                    for j in range(2):
                        jj = h * 2 + j
                        nc.tensor.matmul(
                            p2[:, j * 512 : (j + 1) * 512],
                            W[64 : 64 + K2, m * 128 : (m + 1) * 128],
                            R[64 : 64 + K2, jj * 512 : (jj + 1) * 512],
                            start=True,
                            stop=True,
                            tile_position=(64, 0),
                        )
                    nc.scalar.activation(
                        e2[:, h * HN : (h + 1) * HN],
                        p2[:],
                        mybir.ActivationFunctionType.Exp,
                    )
                return e2

            def mm_p1(p1h, m, b, h, j):
                col = (b * MT + m) * 128
                jj = h * 2 + j
                nc.tensor.matmul(
                    p1h[:, j * 512 : (j + 1) * 512],
                    W[32 : 32 + K1, col : col + 128],
                    R[32 : 32 + K1, b * NS + jj * 512 : b * NS + (jj + 1) * 512],
                    start=True,
                    stop=True,
                    tile_position=(32, 0),
                )

            def mm_pc(pch, m, b, h, j):
                col = (b * MT + m) * 128
                jj = h * 2 + j
                nc.tensor.matmul(
                    pch[:, j * 512 : (j + 1) * 512],
                    W[0:KC, col : col + 128],
                    R[0:KC, b * NS + jj * 512 : b * NS + (jj + 1) * 512],
                    start=True,
                    stop=True,
                    tile_position=(0, 0),
                )

            # Software pipeline: the s-pipeline (p1 matmuls -> exp -> add)
            # for tile t+1 runs concurrently with the pC-pipeline
            # (pC matmuls -> mult -> DMA) of tile t, so the multiply fires as
            # soon as pC lands and pC's PSUM residency stays short.
            e2_cur = None
            s_prev = None
            o = None

            def emit_A(t):
                """s-pipeline for tile t: p1 mms, exps, adds -> returns s."""
                nonlocal e2_cur
                m, b = divmod(t, B)
                if b == 0:
                    e2_cur = emit_e2(m)
                e1 = e1pool.tile([128, NS], F16, tag="e1")
                s = spool.tile([128, NS], F16, tag="s")
                p1h = [
                    p1pool.tile([128, HN], F32, tag="p1", name=f"p1h{h}")
                    for h in range(2)
                ]
                for h in range(2):
                    for j in range(2):
                        mm_p1(p1h[h], m, b, h, j)
                    nc.scalar.activation(
                        e1[:, h * HN : (h + 1) * HN],
                        p1h[h][:],
                        mybir.ActivationFunctionType.Exp,
                    )
                for h in range(2):
                    hs = slice(h * HN, (h + 1) * HN)
                    if _add_on_gps(t, h):
                        nc.gpsimd.tensor_add(s[:, hs], e1[:, hs], e2_cur[:, hs])
                    else:
                        nc.vector.tensor_add(s[:, hs], e1[:, hs], e2_cur[:, hs])
                return s

            s_prev = emit_A(0)
            for t in range(NT_TILES):
                m, b = divmod(t, B)
                if b % 2 == 0:
                    o = opool.tile([128, 2 * NS], F16, tag="o")
                oc = (b % 2) * NS
                pch = [
                    pcpool.tile([128, HN], F32, tag="pc", name=f"pch{h}")
                    for h in range(2)
                ]
                for h in range(2):
                    for j in range(2):
                        mm_pc(pch[h], m, b, h, j)
                # multiplies for tile t (s_prev is ready from the last step)
                for h in range(2):
                    nc.vector.tensor_tensor(
                        o[:, oc + h * HN : oc + (h + 1) * HN],
                        pch[h][:],
                        s_prev[:, h * HN : (h + 1) * HN],
                        mybir.AluOpType.mult,
                    )
                if t + 1 < NT_TILES:
                    s_prev = emit_A(t + 1)
                if b % 2 == 1:
                    nc.sync.dma_start(
                        out[m, :, (b - 1) * NS : (b + 1) * NS], o[:]
                    )

    _split_multiwait_insts(nc)
    return nc


# ---------------------------------------------------------------------------
# Host-side input prep
# ---------------------------------------------------------------------------

def _f16(x):
    return np.asarray(x, dtype=np.float16)


def _hi_lo(x):
    """Split fp64 vector into two fp16 rows summing to ~x."""
    hi = _f16(x)
    lo = _f16(x - hi.astype(np.float64))
    return hi, lo


def prepare_inputs(guidance, clusters, coords):
    ci = np.asarray(coords[0], dtype=np.int64)
    cj = np.asarray(coords[1], dtype=np.int64)
    # gathers: [B, C, NS]
    sel_g = guidance[:, :, ci, cj].astype(np.float64)
    sel_c = clusters[:, :, ci, cj].astype(np.float32)

    # --- cluster Gram operands (fp16 snap) ---
    c16 = _f16(sel_c)  # [B, 27, NS] rhs
    wc_all = -c16  # lhsT (negated -> folds the leading minus)

    # --- first-exp argument operands ---
    # arg1 = -cd/(2a) - gd/(2beta) + ln(W1) ; 2a = 1
    u16 = _f16(sel_g / np.sqrt(2.0 * BETA))  # [B, 3, NS]
    xc16 = _f16((np.stack([ci, cj]) - 112.0))  # [2, NS] exact
    f1 = (u16.astype(np.float64) ** 2).sum(1) + (
        xc16.astype(np.float64) ** 2
    ).sum(0)  # [B, NS]
    a1_all = np.empty((B, K1, NS), np.float16)
    r1_all = np.empty((B, K1, NS), np.float16)
    ones = np.ones(NS, np.float16)
    for b in range(B):
        b1 = np.log(W1) - f1[b]
        b1h, b1l = _hi_lo(b1)
        f1h, f1l = _hi_lo(f1[b])
        a1_all[b, 0:3] = u16[b]
        a1_all[b, 3:5] = xc16
        a1_all[b, 5] = ones
        a1_all[b, 6] = ones
        a1_all[b, 7] = f1h
        a1_all[b, 8] = f1l
        r1_all[b, 0:3] = _f16(2.0 * u16[b].astype(np.float64))
        r1_all[b, 3:5] = _f16(2.0 * xc16.astype(np.float64))
        r1_all[b, 5] = b1h
        r1_all[b, 6] = b1l
        r1_all[b, 7] = -ones
        r1_all[b, 8] = -ones

    # --- second-exp argument operands (batch independent) ---
    v = (np.stack([ci, cj]) - 112.0) / np.sqrt(2.0 * GAMMA)  # [2, NS] fp64
    vh = _f16(v)
    vl = _f16(v - vh.astype(np.float64))
    vs = vh.astype(np.float64) + vl.astype(np.float64)  # snapped value
    f2 = (vs**2).sum(0)  # [NS]
    b2 = np.log(W2) - f2
    b2h, b2l = _hi_lo(b2)
    f2h, f2l = _hi_lo(f2)
    a2 = np.empty((K2, NS), np.float16)
    r2 = np.empty((K2, NS), np.float16)
    # cross products: (vh+vl)_n * 2*(vh+vl)_m  per dim
    a2[0:2] = vh
    a2[2:4] = vh
    a2[4:6] = vl
    a2[6:8] = vl
    r2[0:2] = _f16(2.0 * vh.astype(np.float64))
    r2[2:4] = _f16(2.0 * vl.astype(np.float64))
    r2[4:6] = _f16(2.0 * vh.astype(np.float64))
    r2[6:8] = _f16(2.0 * vl.astype(np.float64))
    a2[8] = ones
    a2[9] = ones
    a2[10] = f2h
    a2[11] = f2l
    r2[8] = b2h
    r2[9] = b2l
    r2[10] = -ones
    r2[11] = -ones

    # --- per-core input maps (core k owns n-rows [256k, 256k+256)) ---
    in_maps = []
    for k in range(NCORES):
        rows = slice(256 * k, 256 * k + 256)
        # A-side column layouts: [(b * MT + m) * 128] for wc/a1, [m * 128] for a2
        wc_k = wc_all[:, :, rows].transpose(1, 0, 2).reshape(KC, B * 256)
        a1_k = a1_all[:, :, rows].transpose(1, 0, 2).reshape(K1, B * 256)
        a2_k = np.ascontiguousarray(a2[:, rows])
        in_maps.append(
            {
                "wc": np.ascontiguousarray(wc_k),
                "a1": np.ascontiguousarray(a1_k),
                "a2": a2_k,
                "rc": np.ascontiguousarray(c16.transpose(1, 0, 2).reshape(KC, B * NS)),
                "r1": np.ascontiguousarray(
                    r1_all.transpose(1, 0, 2).reshape(K1, B * NS)
                ),
                "r2": r2,
            }
        )
    return in_maps


_NC_CACHE = {}


def _get_nc():
    if "nc" not in _NC_CACHE:
        _NC_CACHE["nc"] = build_nc()
    return _NC_CACHE["nc"]


def kernel(guidance, clusters, coords):
    guidance = np.asarray(guidance)
    clusters = np.asarray(clusters)
    coords = np.asarray(coords)
    in_maps = prepare_inputs(guidance, clusters, coords)
    nc = _get_nc()
    res = bass_utils.run_bass_kernel_spmd(nc, in_maps, list(range(NCORES)))
    # res.results[k]["out"]: [MT, 128, B*NS] fp16 -> rows 256k..256k+256
    full = np.concatenate(
        [
            np.asarray(res.results[k]["out"])
            .reshape(MT, 128, B, NS)
            .transpose(2, 0, 1, 3)
            .reshape(B, MT * 128, NS)
            for k in range(NCORES)
        ],
        axis=1,
    )
    return full.astype(np.float32)


# revision 11
# speedup vs baseline: 1.0656x; 1.0019x over previous
"""Trainium2 Bass kernel for nn_ContrastiveCRFLoss (self-contained).

Math: for each batch b and sample pairs (n, m) over 2048 gathered pixels:
    out[b,n,m] = -(C[b,n,m] * (W1*exp(-cd - gd[b]/(2*BETA)) + W2*exp(-cd/(2*GAMMA))))
where C = cluster Gram, cd = squared coord distance, gd = squared guidance
distance.

Device strategy (8 cores, grid-parallel over the n-rows):
  - Each core owns a 256-row block of the 2048x2048 pair grid, all 8 batches.
  - Three small-K fp16 matmuls per output tile, packed into PE row groups at
    partitions 0 / 32 / 64:
      group0 K=27: pC  = (-clusters)^T clusters            (negated Gram)
      group1 K=9 : p1  = full argument of the first exp    (augmented Gram)
      group2 K=12: p2  = full argument of the second exp   (coord-only, shared
                                                            across batches)
  - ACT: e1 = exp(p1) -> fp16 SBUF at N=2048 (PSUM p1 spans 4 banks).
    pC lands in 2 rotating [128,1024] PSUM tiles (the other 4 banks).
  - DVE: s = e1 + e2 at N=2048 fp16 (2x mode) for a subset of tiles; GpSimd
    handles the rest of the adds.  DVE does all multiplies o = pC * s from
    PSUM (1x mode, N=1024 per pC half), writing fp16.
  - Output fp16 [MT, 128, B*NS]: batches adjacent in the free dim so a
    two-batch [128, 2*NS] DMA writes 8KB contiguous per partition row.
"""

import numpy as np

import concourse.bass as bass
import concourse.mybir as mybir
import concourse.bass_utils as bass_utils
from concourse.tile import TileContext
from concourse.vector_clock import ScopedClock

F16 = mybir.dt.float16
F32 = mybir.dt.float32

# problem constants (hardcoded per the task contract)
ALPHA, BETA, GAMMA = 0.5, 0.15, 25.0
W1, W2, SHIFT = 10.0, 3.0, 0.0
B, CG, CC, H = 8, 3, 27, 224
NS = 2048  # samples
NCORES = 8
MT = 2  # 128-row M-tiles per core
KC, K1, K2 = 27, 9, 12
HN = NS // 2  # 1024

# half-tiles (t, h) whose e1+e2 add runs on GpSimd instead of DVE: GpSimd
# owns h=0 of every tile plus h=1 of four tiles; DVE owns the rest plus all
# multiplies
def _add_on_gps(t, h):
    return h == 0

# ---------------------------------------------------------------------------
# Walrus in this image rejects >1 sync wait per instruction. Split the Tile
# tail-drain's waits and any multi-wait instruction into single-wait NOPs.
# ---------------------------------------------------------------------------
_MAXW = 1


def _split_drain_and_barrier(self, tick_clock, wait_clock):
    probe = self.nc.sync.nop(nofuse=True)
    wait_clock.add_sem_waits(probe.ins, ScopedClock({None: tick_clock.global_clock}))
    si = probe.ins.sync_info
    waits = list(si.on_wait)
    probe.ins.sync_info = mybir.SyncInfo(
        on_wait=waits[:_MAXW], on_update=list(si.on_update)
    )
    for i in range(_MAXW, len(waits), _MAXW):
        n2 = self.nc.sync.nop(nofuse=True)
        n2.ins.sync_info = mybir.SyncInfo(on_wait=waits[i : i + _MAXW], on_update=[])
    self.nc.sync.drain()
    self.nc.all_engine_barrier()
    popped = self.nc._tile_sem_poison_stack.pop()
    assert popped is self._sem_poison
    self.nc.clear_and_free_semaphores(list(self.sems.allocated().values()))
    self.nc.all_engine_barrier()


def _split_multiwait_insts(nc):
    n_split = 0
    for fn in nc.m.functions:
        for bb in fn.blocks:
            insts = list(bb.instructions)
            new_insts = []
            changed = False
            for inst in insts:
                si = inst.sync_info
                waits = list(si.on_wait) if si is not None else []
                if len(waits) > _MAXW:
                    n_split += 1
                    changed = True
                    n_extra = len(waits) - _MAXW
                    for i in range(0, n_extra, _MAXW):
                        nop = mybir.InstNoOp(
                            name=nc.get_next_instruction_name(),
                            engine=inst.engine,
                            bass_nofuse=True,
                            sync_info=mybir.SyncInfo(
                                on_wait=waits[i : i + _MAXW], on_update=[]
                            ),
                        )
                        new_insts.append(nop)
                    inst.sync_info = mybir.SyncInfo(
                        on_wait=waits[n_extra:], on_update=list(si.on_update)
                    )
                new_insts.append(inst)
            if changed:
                bb.instructions = new_insts
    return n_split


def _install_tile_patch():
    TileContext._drain_and_barrier = _split_drain_and_barrier


# ---------------------------------------------------------------------------
# Device program (identical on all cores; data differs per core)
# ---------------------------------------------------------------------------

def build_nc():
    _install_tile_patch()
    nc = bass.Bass()
    wc = nc.declare_dram_parameter("wc", [KC, MT * B * 128], F16, isOutput=False)
    a1 = nc.declare_dram_parameter("a1", [K1, MT * B * 128], F16, isOutput=False)
    a2 = nc.declare_dram_parameter("a2", [K2, MT * 128], F16, isOutput=False)
    rc = nc.declare_dram_parameter("rc", [KC, B * NS], F16, isOutput=False)
    r1 = nc.declare_dram_parameter("r1", [K1, B * NS], F16, isOutput=False)
    r2 = nc.declare_dram_parameter("r2", [K2, NS], F16, isOutput=False)
    out = nc.declare_dram_parameter("out", [MT, 128, B * NS], F16, isOutput=True)

    with TileContext(nc) as tc:
        with (
            tc.tile_pool(name="w", bufs=1) as wpool,
            tc.tile_pool(name="r", bufs=1) as rpool,
            tc.tile_pool(name="e2p", bufs=2) as e2pool,
            tc.tile_pool(name="e1p", bufs=3) as e1pool,
            tc.tile_pool(name="sp", bufs=3) as spool,
            tc.tile_pool(name="ob", bufs=3) as opool,
            tc.tile_pool(name="pc", bufs=2, space="PSUM") as pcpool,
            tc.tile_pool(name="p1", bufs=2, space="PSUM") as p1pool,
        ):
            # Per-batch W/R tiles with per-batch DMAs: tile 0's matmuls only
            # wait for batch 0's inputs, so compute starts ~10us earlier than
            # with monolithic loads.  a2/r2 (tile 0's e2 inputs) go first.
            Wa2 = wpool.tile([128, MT * 128], F16, name="wa2")
            Rr2 = rpool.tile([128, NS], F16, name="rr2")
            nc.sync.dma_start(Wa2[64 : 64 + K2, :], a2[:])
            nc.sync.dma_start(Rr2[64 : 64 + K2, :], r2[:])
            Wb = []
            Rb = []
            for b in range(B):
                wt = wpool.tile([128, MT * 128], F16, name=f"w{b}")
                rt = rpool.tile([128, NS], F16, name=f"r{b}")
                cs = slice(b * MT * 128, (b + 1) * MT * 128)
                rs = slice(b * NS, (b + 1) * NS)
                nc.sync.dma_start(rt[0:KC, :], rc[:, rs])
                nc.sync.dma_start(rt[32 : 32 + K1, :], r1[:, rs])
                nc.sync.dma_start(wt[0:KC, :], wc[:, cs])
                nc.sync.dma_start(wt[32 : 32 + K1, :], a1[:, cs])
                Wb.append(wt)
                Rb.append(rt)

            NT_TILES = MT * B  # 16

            def emit_e2(m):
                """p2 matmuls + exps for the batch-independent term of m."""
                e2 = e2pool.tile([128, NS], F16, tag="e2")
                for h in range(2):
                    p2 = p1pool.tile([128, HN], F32, tag="p1")
                    for j in range(2):
                        jj = h * 2 + j
                        nc.tensor.matmul(
                            p2[:, j * 512 : (j + 1) * 512],
                            Wa2[64 : 64 + K2, m * 128 : (m + 1) * 128],
                            Rr2[64 : 64 + K2, jj * 512 : (jj + 1) * 512],
                            start=True,
                            stop=True,
                            tile_position=(64, 0),
                        )
                    nc.scalar.activation(
                        e2[:, h * HN : (h + 1) * HN],
                        p2[:],
                        mybir.ActivationFunctionType.Exp,
                    )
                return e2

            def mm_p1(p1h, m, b, h, j):
                col = m * 128
                jj = h * 2 + j
                nc.tensor.matmul(
                    p1h[:, j * 512 : (j + 1) * 512],
                    Wb[b][32 : 32 + K1, col : col + 128],
                    Rb[b][32 : 32 + K1, jj * 512 : (jj + 1) * 512],
                    start=True,
                    stop=True,
                    tile_position=(32, 0),
                )

            def mm_pc(pch, m, b, h, j):
                col = m * 128
                jj = h * 2 + j
                nc.tensor.matmul(
                    pch[:, j * 512 : (j + 1) * 512],
                    Wb[b][0:KC, col : col + 128],
                    Rb[b][0:KC, jj * 512 : (jj + 1) * 512],
                    start=True,
                    stop=True,
                    tile_position=(0, 0),
                )

            # Software pipeline: the s-pipeline (p1 matmuls -> exp -> add)
            # for tile t+1 runs concurrently with the pC-pipeline
            # (pC matmuls -> mult -> DMA) of tile t, so the multiply fires as
            # soon as pC lands and pC's PSUM residency stays short.
            e2_cur = None
            s_prev = None
            o = None

            def emit_A(t):
                """s-pipeline for tile t: p1 mms, exps, adds -> returns s."""
                nonlocal e2_cur
                m, b = divmod(t, B)
                if b == 0:
                    e2_cur = emit_e2(m)
                e1 = e1pool.tile([128, NS], F16, tag="e1")
                s = spool.tile([128, NS], F16, tag="s")
                p1h = [
                    p1pool.tile([128, HN], F32, tag="p1", name=f"p1h{h}")
                    for h in range(2)
                ]
                for h in range(2):
                    for j in range(2):
                        mm_p1(p1h[h], m, b, h, j)
                    nc.scalar.activation(
                        e1[:, h * HN : (h + 1) * HN],
                        p1h[h][:],
                        mybir.ActivationFunctionType.Exp,
                    )
                for h in range(2):
                    hs = slice(h * HN, (h + 1) * HN)
                    if _add_on_gps(t, h):
                        nc.gpsimd.tensor_add(s[:, hs], e1[:, hs], e2_cur[:, hs])
                    else:
                        nc.vector.tensor_add(s[:, hs], e1[:, hs], e2_cur[:, hs])
                return s

            s_prev = emit_A(0)
            for t in range(NT_TILES):
                m, b = divmod(t, B)
                if b % 2 == 0:
                    o = opool.tile([128, 2 * NS], F16, tag="o")
                oc = (b % 2) * NS
                pch = [
                    pcpool.tile([128, HN], F32, tag="pc", name=f"pch{h}")
                    for h in range(2)
                ]
                for h in range(2):
                    for j in range(2):
                        mm_pc(pch[h], m, b, h, j)
                # multiplies for tile t (s_prev is ready from the last step)
                for h in range(2):
                    nc.vector.tensor_tensor(
                        o[:, oc + h * HN : oc + (h + 1) * HN],
                        pch[h][:],
                        s_prev[:, h * HN : (h + 1) * HN],
                        mybir.AluOpType.mult,
                    )
                if t + 1 < NT_TILES:
                    s_prev = emit_A(t + 1)
                if b % 2 == 1:
                    nc.sync.dma_start(
                        out[m, :, (b - 1) * NS : (b + 1) * NS], o[:]
                    )

    _split_multiwait_insts(nc)
    return nc


# ---------------------------------------------------------------------------
# Host-side input prep
# ---------------------------------------------------------------------------

def _f16(x):
    return np.asarray(x, dtype=np.float16)


def _hi_lo(x):
    """Split fp64 vector into two fp16 rows summing to ~x."""
    hi = _f16(x)
    lo = _f16(x - hi.astype(np.float64))
    return hi, lo


def prepare_inputs(guidance, clusters, coords):
    ci = np.asarray(coords[0], dtype=np.int64)
    cj = np.asarray(coords[1], dtype=np.int64)
    # gathers: [B, C, NS]
    sel_g = guidance[:, :, ci, cj].astype(np.float64)
    sel_c = clusters[:, :, ci, cj].astype(np.float32)

    # --- cluster Gram operands (fp16 snap) ---
    c16 = _f16(sel_c)  # [B, 27, NS] rhs
    wc_all = -c16  # lhsT (negated -> folds the leading minus)

    # --- first-exp argument operands ---
    # arg1 = -cd/(2a) - gd/(2beta) + ln(W1) ; 2a = 1
    u16 = _f16(sel_g / np.sqrt(2.0 * BETA))  # [B, 3, NS]
    xc16 = _f16((np.stack([ci, cj]) - 112.0))  # [2, NS] exact
    f1 = (u16.astype(np.float64) ** 2).sum(1) + (
        xc16.astype(np.float64) ** 2
    ).sum(0)  # [B, NS]
    a1_all = np.empty((B, K1, NS), np.float16)
    r1_all = np.empty((B, K1, NS), np.float16)
    ones = np.ones(NS, np.float16)
    for b in range(B):
        b1 = np.log(W1) - f1[b]
        b1h, b1l = _hi_lo(b1)
        f1h, f1l = _hi_lo(f1[b])
        a1_all[b, 0:3] = u16[b]
        a1_all[b, 3:5] = xc16
        a1_all[b, 5] = ones
        a1_all[b, 6] = ones
        a1_all[b, 7] = f1h
        a1_all[b, 8] = f1l
        r1_all[b, 0:3] = _f16(2.0 * u16[b].astype(np.float64))
        r1_all[b, 3:5] = _f16(2.0 * xc16.astype(np.float64))
        r1_all[b, 5] = b1h
        r1_all[b, 6] = b1l
        r1_all[b, 7] = -ones
        r1_all[b, 8] = -ones

    # --- second-exp argument operands (batch independent) ---
    v = (np.stack([ci, cj]) - 112.0) / np.sqrt(2.0 * GAMMA)  # [2, NS] fp64
    vh = _f16(v)
    vl = _f16(v - vh.astype(np.float64))
    vs = vh.astype(np.float64) + vl.astype(np.float64)  # snapped value
    f2 = (vs**2).sum(0)  # [NS]
    b2 = np.log(W2) - f2
    b2h, b2l = _hi_lo(b2)
    f2h, f2l = _hi_lo(f2)
    a2 = np.empty((K2, NS), np.float16)
    r2 = np.empty((K2, NS), np.float16)
    # cross products: (vh+vl)_n * 2*(vh+vl)_m  per dim
    a2[0:2] = vh
    a2[2:4] = vh
    a2[4:6] = vl
    a2[6:8] = vl
    r2[0:2] = _f16(2.0 * vh.astype(np.float64))
    r2[2:4] = _f16(2.0 * vl.astype(np.float64))
    r2[4:6] = _f16(2.0 * vh.astype(np.float64))
    r2[6:8] = _f16(2.0 * vl.astype(np.float64))
    a2[8] = ones
    a2[9] = ones
    a2[10] = f2h
    a2[11] = f2l
    r2[8] = b2h
    r2[9] = b2l
    r2[10] = -ones
    r2[11] = -ones

    # --- per-core input maps (core k owns n-rows [256k, 256k+256)) ---
    in_maps = []
    for k in range(NCORES):
        rows = slice(256 * k, 256 * k + 256)
        # A-side column layouts: [(b * MT + m) * 128] for wc/a1, [m * 128] for a2
        wc_k = wc_all[:, :, rows].transpose(1, 0, 2).reshape(KC, B * 256)
        a1_k = a1_all[:, :, rows].transpose(1, 0, 2).reshape(K1, B * 256)
        a2_k = np.ascontiguousarray(a2[:, rows])
        in_maps.append(
            {
                "wc": np.ascontiguousarray(wc_k),
                "a1": np.ascontiguousarray(a1_k),
                "a2": a2_k,
                "rc": np.ascontiguousarray(c16.transpose(1, 0, 2).reshape(KC, B * NS)),
                "r1": np.ascontiguousarray(
                    r1_all.transpose(1, 0, 2).reshape(K1, B * NS)
                ),
                "r2": r2,
            }
        )
    return in_maps


_NC_CACHE = {}


def _get_nc():
    if "nc" not in _NC_CACHE:
        _NC_CACHE["nc"] = build_nc()
    return _NC_CACHE["nc"]


def kernel(guidance, clusters, coords):
    guidance = np.asarray(guidance)
    clusters = np.asarray(clusters)
    coords = np.asarray(coords)
    in_maps = prepare_inputs(guidance, clusters, coords)
    nc = _get_nc()
    res = bass_utils.run_bass_kernel_spmd(nc, in_maps, list(range(NCORES)))
    # res.results[k]["out"]: [MT, 128, B*NS] fp16 -> rows 256k..256k+256
    full = np.concatenate(
        [
            np.asarray(res.results[k]["out"])
            .reshape(MT, 128, B, NS)
            .transpose(2, 0, 1, 3)
            .reshape(B, MT * 128, NS)
            for k in range(NCORES)
        ],
        axis=1,
    )
    return full.astype(np.float32)


# revision 13
# speedup vs baseline: 1.1457x; 1.0752x over previous
"""Trainium2 Bass kernel for nn_ContrastiveCRFLoss (self-contained).

Math: for each batch b and sample pairs (n, m) over 2048 gathered pixels:
    out[b,n,m] = -(C[b,n,m] * (W1*exp(-cd - gd[b]/(2*BETA)) + W2*exp(-cd/(2*GAMMA))))
where C = cluster Gram, cd = squared coord distance, gd = squared guidance
distance.

Device strategy (8 cores, grid-parallel over the n-rows):
  - Each core owns a 256-row block of the 2048x2048 pair grid, all 8 batches.
  - Three small-K fp16 matmuls per output tile, packed into PE row groups at
    partitions 0 / 32 / 64:
      group0 K=27: pC  = (-clusters)^T clusters            (negated Gram)
      group1 K=9 : p1  = full argument of the first exp    (augmented Gram)
      group2 K=12: p2  = full argument of the second exp   (coord-only, shared
                                                            across batches)
  - ACT: e1 = exp(p1) -> fp16 SBUF at N=2048 (PSUM p1 spans 4 banks).
    pC lands in 2 rotating [128,1024] PSUM tiles (the other 4 banks).
  - DVE: s = e1 + e2 at N=2048 fp16 (2x mode) for a subset of tiles; GpSimd
    handles the rest of the adds.  DVE does all multiplies o = pC * s from
    PSUM (1x mode, N=1024 per pC half), writing fp16.
  - Output fp16 [MT, 128, B*NS]: batches adjacent in the free dim so a
    two-batch [128, 2*NS] DMA writes 8KB contiguous per partition row.
"""

import numpy as np

import concourse.bass as bass
import concourse.mybir as mybir
import concourse.bass_utils as bass_utils
from concourse.tile import TileContext
from concourse.vector_clock import ScopedClock

F16 = mybir.dt.float16
F32 = mybir.dt.float32

# problem constants (hardcoded per the task contract)
ALPHA, BETA, GAMMA = 0.5, 0.15, 25.0
W1, W2, SHIFT = 10.0, 3.0, 0.0
B, CG, CC, H = 8, 3, 27, 224
NS = 2048  # samples
NCORES = 8
MT = 2  # 128-row M-tiles per core
KC, K1, K2 = 27, 9, 12
HN = NS // 2  # 1024

# half-tiles (t, h) whose e1+e2 add runs on GpSimd instead of DVE: GpSimd
# owns h=0 of every tile plus h=1 of four tiles; DVE owns the rest plus all
# multiplies
def _add_on_gps(t, h):
    return h == 0

# ---------------------------------------------------------------------------
# Walrus in this image rejects >1 sync wait per instruction. Split the Tile
# tail-drain's waits and any multi-wait instruction into single-wait NOPs.
# ---------------------------------------------------------------------------
_MAXW = 1


def _split_drain_and_barrier(self, tick_clock, wait_clock):
    probe = self.nc.sync.nop(nofuse=True)
    wait_clock.add_sem_waits(probe.ins, ScopedClock({None: tick_clock.global_clock}))
    si = probe.ins.sync_info
    waits = list(si.on_wait)
    probe.ins.sync_info = mybir.SyncInfo(
        on_wait=waits[:_MAXW], on_update=list(si.on_update)
    )
    for i in range(_MAXW, len(waits), _MAXW):
        n2 = self.nc.sync.nop(nofuse=True)
        n2.ins.sync_info = mybir.SyncInfo(on_wait=waits[i : i + _MAXW], on_update=[])
    self.nc.sync.drain()
    self.nc.all_engine_barrier()
    popped = self.nc._tile_sem_poison_stack.pop()
    assert popped is self._sem_poison
    self.nc.clear_and_free_semaphores(list(self.sems.allocated().values()))
    self.nc.all_engine_barrier()


def _split_multiwait_insts(nc):
    n_split = 0
    for fn in nc.m.functions:
        for bb in fn.blocks:
            insts = list(bb.instructions)
            new_insts = []
            changed = False
            for inst in insts:
                si = inst.sync_info
                waits = list(si.on_wait) if si is not None else []
                if len(waits) > _MAXW:
                    n_split += 1
                    changed = True
                    n_extra = len(waits) - _MAXW
                    for i in range(0, n_extra, _MAXW):
                        nop = mybir.InstNoOp(
                            name=nc.get_next_instruction_name(),
                            engine=inst.engine,
                            bass_nofuse=True,
                            sync_info=mybir.SyncInfo(
                                on_wait=waits[i : i + _MAXW], on_update=[]
                            ),
                        )
                        new_insts.append(nop)
                    inst.sync_info = mybir.SyncInfo(
                        on_wait=waits[n_extra:], on_update=list(si.on_update)
                    )
                new_insts.append(inst)
            if changed:
                bb.instructions = new_insts
    return n_split


def _install_tile_patch():
    TileContext._drain_and_barrier = _split_drain_and_barrier


# ---------------------------------------------------------------------------
# Device program (identical on all cores; data differs per core)
# ---------------------------------------------------------------------------

def build_nc():
    _install_tile_patch()
    nc = bass.Bass()
    wc = nc.declare_dram_parameter("wc", [KC, MT * B * 128], F16, isOutput=False)
    a1 = nc.declare_dram_parameter("a1", [K1, MT * B * 128], F16, isOutput=False)
    a2 = nc.declare_dram_parameter("a2", [K2, MT * 128], F16, isOutput=False)
    rc = nc.declare_dram_parameter("rc", [KC, B * NS], F16, isOutput=False)
    r1 = nc.declare_dram_parameter("r1", [K1, B * NS], F16, isOutput=False)
    r2 = nc.declare_dram_parameter("r2", [K2, NS], F16, isOutput=False)
    out = nc.declare_dram_parameter("out", [MT, 128, B * NS], F16, isOutput=True)

    with TileContext(nc) as tc:
        with (
            tc.tile_pool(name="w", bufs=1) as wpool,
            tc.tile_pool(name="r", bufs=1) as rpool,
            tc.tile_pool(name="e2p", bufs=2) as e2pool,
            tc.tile_pool(name="e1p", bufs=3) as e1pool,
            tc.tile_pool(name="sp", bufs=3) as spool,
            tc.tile_pool(name="ob", bufs=3) as opool,
            tc.tile_pool(name="pc", bufs=2, space="PSUM") as pcpool,
            tc.tile_pool(name="p1", bufs=2, space="PSUM") as p1pool,
        ):
            # Split input loads so tile 0's inputs land first, but keep the
            # dispatch count low: each DMA_DIRECT2D costs ~710ns on the Sync
            # queue and delays the output-DMA dispatches queued behind it.
            Wa2 = wpool.tile([128, MT * 128], F16, name="wa2")
            Rr2 = rpool.tile([128, NS], F16, name="rr2")
            W = wpool.tile([128, MT * B * 128], F16, name="w")
            Rp = [
                rpool.tile([128, 2 * NS], F16, name=f"r{p}") for p in range(B // 2)
            ]
            nc.sync.dma_start(Wa2[64 : 64 + K2, :], a2[:])
            nc.sync.dma_start(Rr2[64 : 64 + K2, :], r2[:])
            nc.sync.dma_start(Rp[0][0:KC, :], rc[:, 0 : 2 * NS])
            nc.sync.dma_start(Rp[0][32 : 32 + K1, :], r1[:, 0 : 2 * NS])
            nc.sync.dma_start(W[0:KC, :], wc[:])
            nc.sync.dma_start(W[32 : 32 + K1, :], a1[:])
            for p in range(1, B // 2):
                rs = slice(p * 2 * NS, (p + 1) * 2 * NS)
                nc.sync.dma_start(Rp[p][0:KC, :], rc[:, rs])
                nc.sync.dma_start(Rp[p][32 : 32 + K1, :], r1[:, rs])

            NT_TILES = MT * B  # 16

            def emit_e2(m):
                """p2 matmuls + exps for the batch-independent term of m."""
                e2 = e2pool.tile([128, NS], F16, tag="e2")
                for h in range(2):
                    p2 = p1pool.tile([128, HN], F32, tag="p1")
                    for j in range(2):
                        jj = h * 2 + j
                        nc.tensor.matmul(
                            p2[:, j * 512 : (j + 1) * 512],
                            Wa2[64 : 64 + K2, m * 128 : (m + 1) * 128],
                            Rr2[64 : 64 + K2, jj * 512 : (jj + 1) * 512],
                            start=True,
                            stop=True,
                            tile_position=(64, 0),
                        )
                    nc.scalar.activation(
                        e2[:, h * HN : (h + 1) * HN],
                        p2[:],
                        mybir.ActivationFunctionType.Exp,
                    )
                return e2

            def mm_p1(p1h, m, b, h, j):
                col = (b * MT + m) * 128
                rcol = (b % 2) * NS + (h * 2 + j) * 512
                nc.tensor.matmul(
                    p1h[:, j * 512 : (j + 1) * 512],
                    W[32 : 32 + K1, col : col + 128],
                    Rp[b // 2][32 : 32 + K1, rcol : rcol + 512],
                    start=True,
                    stop=True,
                    tile_position=(32, 0),
                )

            def mm_pc(pch, m, b, h, j):
                col = (b * MT + m) * 128
                rcol = (b % 2) * NS + (h * 2 + j) * 512
                nc.tensor.matmul(
                    pch[:, j * 512 : (j + 1) * 512],
                    W[0:KC, col : col + 128],
                    Rp[b // 2][0:KC, rcol : rcol + 512],
                    start=True,
                    stop=True,
                    tile_position=(0, 0),
                )

            # Software pipeline: the s-pipeline (p1 matmuls -> exp -> add)
            # for tile t+1 runs concurrently with the pC-pipeline
            # (pC matmuls -> mult -> DMA) of tile t, so the multiply fires as
            # soon as pC lands and pC's PSUM residency stays short.
            e2_cur = None
            s_prev = None
            o = None

            def emit_A(t):
                """s-pipeline for tile t: p1 mms, exps, adds -> returns s."""
                nonlocal e2_cur
                m, b = divmod(t, B)
                if b == 0:
                    e2_cur = emit_e2(m)
                e1 = e1pool.tile([128, NS], F16, tag="e1")
                s = spool.tile([128, NS], F16, tag="s")
                p1h = [
                    p1pool.tile([128, HN], F32, tag="p1", name=f"p1h{h}")
                    for h in range(2)
                ]
                for h in range(2):
                    for j in range(2):
                        mm_p1(p1h[h], m, b, h, j)
                    nc.scalar.activation(
                        e1[:, h * HN : (h + 1) * HN],
                        p1h[h][:],
                        mybir.ActivationFunctionType.Exp,
                    )
                for h in range(2):
                    hs = slice(h * HN, (h + 1) * HN)
                    if _add_on_gps(t, h):
                        nc.gpsimd.tensor_add(s[:, hs], e1[:, hs], e2_cur[:, hs])
                    else:
                        nc.vector.tensor_add(s[:, hs], e1[:, hs], e2_cur[:, hs])
                return s

            s_prev = emit_A(0)
            for t in range(NT_TILES):
                m, b = divmod(t, B)
                if b % 2 == 0:
                    o = opool.tile([128, 2 * NS], F16, tag="o")
                oc = (b % 2) * NS
                pch = [
                    pcpool.tile([128, HN], F32, tag="pc", name=f"pch{h}")
                    for h in range(2)
                ]
                for h in range(2):
                    for j in range(2):
                        mm_pc(pch[h], m, b, h, j)
                # multiplies for tile t (s_prev is ready from the last step)
                for h in range(2):
                    nc.vector.tensor_tensor(
                        o[:, oc + h * HN : oc + (h + 1) * HN],
                        pch[h][:],
                        s_prev[:, h * HN : (h + 1) * HN],
                        mybir.AluOpType.mult,
                    )
                if t + 1 < NT_TILES:
                    s_prev = emit_A(t + 1)
                if b % 2 == 1:
                    nc.sync.dma_start(
                        out[m, :, (b - 1) * NS : (b + 1) * NS], o[:]
                    )

    _split_multiwait_insts(nc)
    return nc


# ---------------------------------------------------------------------------
# Host-side input prep
# ---------------------------------------------------------------------------

def _f16(x):
    return np.asarray(x, dtype=np.float16)


def _hi_lo(x):
    """Split fp64 vector into two fp16 rows summing to ~x."""
    hi = _f16(x)
    lo = _f16(x - hi.astype(np.float64))
    return hi, lo


def prepare_inputs(guidance, clusters, coords):
    ci = np.asarray(coords[0], dtype=np.int64)
    cj = np.asarray(coords[1], dtype=np.int64)
    # gathers: [B, C, NS]
    sel_g = guidance[:, :, ci, cj].astype(np.float64)
    sel_c = clusters[:, :, ci, cj].astype(np.float32)

    # --- cluster Gram operands (fp16 snap) ---
    c16 = _f16(sel_c)  # [B, 27, NS] rhs
    wc_all = -c16  # lhsT (negated -> folds the leading minus)

    # --- first-exp argument operands ---
    # arg1 = -cd/(2a) - gd/(2beta) + ln(W1) ; 2a = 1
    u16 = _f16(sel_g / np.sqrt(2.0 * BETA))  # [B, 3, NS]
    xc16 = _f16((np.stack([ci, cj]) - 112.0))  # [2, NS] exact
    f1 = (u16.astype(np.float64) ** 2).sum(1) + (
        xc16.astype(np.float64) ** 2
    ).sum(0)  # [B, NS]
    a1_all = np.empty((B, K1, NS), np.float16)
    r1_all = np.empty((B, K1, NS), np.float16)
    ones = np.ones(NS, np.float16)
    for b in range(B):
        b1 = np.log(W1) - f1[b]
        b1h, b1l = _hi_lo(b1)
        f1h, f1l = _hi_lo(f1[b])
        a1_all[b, 0:3] = u16[b]
        a1_all[b, 3:5] = xc16
        a1_all[b, 5] = ones
        a1_all[b, 6] = ones
        a1_all[b, 7] = f1h
        a1_all[b, 8] = f1l
        r1_all[b, 0:3] = _f16(2.0 * u16[b].astype(np.float64))
        r1_all[b, 3:5] = _f16(2.0 * xc16.astype(np.float64))
        r1_all[b, 5] = b1h
        r1_all[b, 6] = b1l
        r1_all[b, 7] = -ones
        r1_all[b, 8] = -ones

    # --- second-exp argument operands (batch independent) ---
    v = (np.stack([ci, cj]) - 112.0) / np.sqrt(2.0 * GAMMA)  # [2, NS] fp64
    vh = _f16(v)
    vl = _f16(v - vh.astype(np.float64))
    vs = vh.astype(np.float64) + vl.astype(np.float64)  # snapped value
    f2 = (vs**2).sum(0)  # [NS]
    b2 = np.log(W2) - f2
    b2h, b2l = _hi_lo(b2)
    f2h, f2l = _hi_lo(f2)
    a2 = np.empty((K2, NS), np.float16)
    r2 = np.empty((K2, NS), np.float16)
    # cross products: (vh+vl)_n * 2*(vh+vl)_m  per dim
    a2[0:2] = vh
    a2[2:4] = vh
    a2[4:6] = vl
    a2[6:8] = vl
    r2[0:2] = _f16(2.0 * vh.astype(np.float64))
    r2[2:4] = _f16(2.0 * vl.astype(np.float64))
    r2[4:6] = _f16(2.0 * vh.astype(np.float64))
    r2[6:8] = _f16(2.0 * vl.astype(np.float64))
    a2[8] = ones
    a2[9] = ones
    a2[10] = f2h
    a2[11] = f2l
    r2[8] = b2h
    r2[9] = b2l
    r2[10] = -ones
    r2[11] = -ones

    # --- per-core input maps (core k owns n-rows [256k, 256k+256)) ---
    in_maps = []
    for k in range(NCORES):
        rows = slice(256 * k, 256 * k + 256)
        # A-side column layouts: [(b * MT + m) * 128] for wc/a1, [m * 128] for a2
        wc_k = wc_all[:, :, rows].transpose(1, 0, 2).reshape(KC, B * 256)
        a1_k = a1_all[:, :, rows].transpose(1, 0, 2).reshape(K1, B * 256)
        a2_k = np.ascontiguousarray(a2[:, rows])
        in_maps.append(
            {
                "wc": np.ascontiguousarray(wc_k),
                "a1": np.ascontiguousarray(a1_k),
                "a2": a2_k,
                "rc": np.ascontiguousarray(c16.transpose(1, 0, 2).reshape(KC, B * NS)),
                "r1": np.ascontiguousarray(
                    r1_all.transpose(1, 0, 2).reshape(K1, B * NS)
                ),
                "r2": r2,
            }
        )
    return in_maps


_NC_CACHE = {}


def _get_nc():
    if "nc" not in _NC_CACHE:
        _NC_CACHE["nc"] = build_nc()
    return _NC_CACHE["nc"]


def kernel(guidance, clusters, coords):
    guidance = np.asarray(guidance)
    clusters = np.asarray(clusters)
    coords = np.asarray(coords)
    in_maps = prepare_inputs(guidance, clusters, coords)
    nc = _get_nc()
    res = bass_utils.run_bass_kernel_spmd(nc, in_maps, list(range(NCORES)))
    # res.results[k]["out"]: [MT, 128, B*NS] fp16 -> rows 256k..256k+256
    full = np.concatenate(
        [
            np.asarray(res.results[k]["out"])
            .reshape(MT, 128, B, NS)
            .transpose(2, 0, 1, 3)
            .reshape(B, MT * 128, NS)
            for k in range(NCORES)
        ],
        axis=1,
    )
    return full.astype(np.float32)
